# revision 33
# baseline (speedup 1.0000x reference)
"""Trainium2 Bass kernel for the mca_g2l sparse-attention module (v2).

Head-parallel over 8 cores (1 head each). Weights/biases are baked into the
NEFF as Const tensors (shipped once at model load, not per-execute); each core
dynamic-slices its head's blocks via partition_id. Per-execute inputs are only
the bf16 x^T shard (1MB/core) and cls_score (8KB).

Two collectives total (all bf16):
  AG : AllGather of x^T shards -> full x^T on every core.
  AR : one AllReduce of [attn_avg^T | rawsim_cls^T | rawsim_reg^T |
       outlin_cls partials | outlin_reg partials]. After it, every core
       forms the full masked-exp matrices locally and computes its head's
       ave-branch columns from its own token-major v (support = vTok) with
       local normalizers, and extracts its 256 output-linear columns from
       the summed partials (2C->2C linears are contraction-sharded).

All matmuls run in bf16 with f32 PSUM accumulation; softmax statistics and
final outputs are f32.
"""

import hashlib

import numpy as np
import ml_dtypes

import concourse.bass as bass
import concourse.bacc as bacc
import concourse.mybir as mybir
import concourse.tile as tile
from concourse.masks import make_identity

F32 = mybir.dt.float32
BF16 = mybir.dt.bfloat16
U16 = mybir.dt.uint16
AF = mybir.ActivationFunctionType

N_CORES = 8
N1 = 512
N2 = 2048
C = 1024
HD = 128
SCALE = 25.0
KT = N2 // 128          # 16 key tiles of 128
TT = N2 // 512          # 4 token tiles of 512
CC = C // 128           # 8 contraction chunks
MYK = N2 // N_CORES     # 256 keys/tokens owned per core

ARR = 5 * N2            # AllReduce rows: [avg | rawsim_cls | rawsim_reg | lin_c 2048 | lin_r 2048]

RG = [list(range(N_CORES))]
B = ("cls", "reg")

# timing-experiment knobs (bench_var.py); defaults = production kernel
SHRINK_AG = False
SHRINK_RS = False
OUT_BF16 = False


def _bf16_u16(a: np.ndarray) -> np.ndarray:
    return np.ascontiguousarray(a, np.float32).astype(ml_dtypes.bfloat16).view(np.uint16)


def make_consts(inputs: dict) -> dict[str, np.ndarray]:
    """Weight layouts for Const baking (see build_nc for the index meanings)."""
    W_q = {"cls": np.asarray(inputs["W_q_cls"], np.float32),
           "reg": np.asarray(inputs["W_q_reg"], np.float32)}
    W_kv = {"cls": np.asarray(inputs["W_kv_cls"], np.float32),
            "reg": np.asarray(inputs["W_kv_reg"], np.float32)}
    W_l = {"cls": np.asarray(inputs["W_lin"], np.float32),
           "reg": np.asarray(inputs["W_lin_reg"], np.float32)}
    b_l = {"cls": np.asarray(inputs["b_lin"], np.float32),
           "reg": np.asarray(inputs["b_lin_reg"], np.float32)}

    # WA[h, ib, p, c, m]: projection lhsT for head h: cols m = q|k|v (128 each),
    # contraction row = c*128+p.
    WA = np.zeros((N_CORES, 2, 128, CC, 384), np.uint16)
    # WL[h, ib, p, cj, j, m]: W_lin rows owned by head h (cj=0: x rows
    # h*128+p; cj=1: x_ori rows C+h*128+p), out col j*128+m.
    WL = np.zeros((N_CORES, 2, 128, 2, 16, 128), np.uint16)
    # BIAS[h, ib, p, m]: b[h*256 + m*128 + p]
    BIAS = np.zeros((N_CORES, 2, 128, 2), np.float32)

    for ib, b in enumerate(B):
        for h in range(N_CORES):
            hs = slice(h * HD, (h + 1) * HD)
            wcat = np.concatenate([W_q[b][:, hs], W_kv[b][:, hs],
                                   W_kv[b][:, C + h * HD:C + (h + 1) * HD]], 1)
            WA[h, ib] = _bf16_u16(wcat).reshape(CC, 128, 384).transpose(1, 0, 2)
            wl = np.stack([W_l[b][h * HD:(h + 1) * HD, :],
                           W_l[b][C + h * HD:C + (h + 1) * HD, :]], 1)
            WL[h, ib] = _bf16_u16(wl).reshape(128, 2, 16, 128)
            BIAS[h, ib] = b_l[b][h * 256:(h + 1) * 256].reshape(2, 128).T
    return {"WA": WA, "WL": WL, "BIAS": BIAS}


def build_nc(consts: dict[str, np.ndarray]):
    """Build the SPMD program (identical on every core; Const data shared)."""
    nc = bacc.Bacc("TRN2", target_bir_lowering=False, debug=False,
                   num_devices=N_CORES)

    xin = nc.dram_tensor("xin", [256, N2], U16, kind="ExternalInput")
    score_t = nc.dram_tensor("score", [1, N2], F32, kind="ExternalInput")
    OUTDT = BF16 if OUT_BF16 else F32
    out_t = nc.dram_tensor("out", [768, 512], OUTDT, kind="ExternalOutput")
    o_out = {"cls": out_t.ap()[0:256, :], "reg": out_t.ap()[256:512, :]}
    a_out = {"cls": out_t.ap()[512:640, :], "reg": out_t.ap()[640:768, :]}

    WAc = nc.inline_tensor(consts["WA"], name="WAc")
    WLc = nc.inline_tensor(consts["WL"], name="WLc")
    BIc = nc.inline_tensor(consts["BIAS"], name="BIc")

    with tile.TileContext(nc) as tc:
        pid = nc.partition_id()
        with tc.tile_pool(name="dram", bufs=1, space="DRAM") as dramp, \
             tc.tile_pool(name="const", bufs=1) as constp, \
             tc.tile_pool(name="persist", bufs=1) as persist:

            # ---- internal DRAM for collectives ----
            agx_in = dramp.tile([256, N2], BF16, name="agx_in")
            agx_out = dramp.tile([2 * C, N2], BF16, name="agx_out",
                                 addr_space=("Local" if SHRINK_AG else "Shared"))
            ar_in = dramp.tile([ARR, N1], BF16, name="ar_in")
            ar_out = dramp.tile([ARR, N1], BF16, name="ar_out",
                                addr_space="Shared")

            if not SHRINK_AG:
                nc.sync.dma_start(agx_in[:], xin.ap().bitcast(BF16))
                nc.gpsimd.collective_compute(
                    "AllGather", mybir.AluOpType.bypass, replica_groups=RG,
                    ins=[agx_in.opt()], outs=[agx_out.opt()])
            else:
                # timing variant: tiny AG, then fan its result over agx_out
                agx_in2 = dramp.tile([32, N2], BF16, name="agx_in2")
                agx_out2 = dramp.tile([256, N2], BF16, name="agx_out2",
                                      addr_space="Shared")
                nc.sync.dma_start(agx_in2[:], xin.ap()[0:32, :].bitcast(BF16))
                nc.gpsimd.collective_compute(
                    "AllGather", mybir.AluOpType.bypass, replica_groups=RG,
                    ins=[agx_in2.opt()], outs=[agx_out2.opt()])
                for j in range(8):
                    nc.sync.dma_start(agx_out[j * 256:(j + 1) * 256, :],
                                      agx_out2[:])
            # full x^T, feature-major: [ib][p, c, tok]
            xa = agx_out[:].rearrange("(c two p) n -> two p c n", two=2, p=128)

            # ---- constants ----
            ones_f = constp.tile([128, 1], F32, name="ones_f")
            nc.vector.memset(ones_f[:], 1.0)
            ones = constp.tile([128, 1], BF16, name="ones")
            nc.vector.tensor_copy(ones[:], ones_f[:])
            ident_f = constp.tile([128, 128], F32, name="ident_f")
            make_identity(nc, ident_f[:])
            ident = constp.tile([128, 128], BF16, name="ident")
            nc.vector.tensor_copy(ident[:], ident_f[:])
            score_s = constp.tile([1, N2], F32, name="score_s")
            nc.sync.dma_start(score_s[:], score_t.ap())
            bias_s = {}
            for i, b in enumerate(B):
                bias_s[b] = constp.tile([128, 2], F32, name=f"bias_{b}",
                                        tag=f"bias_{b}")
                nc.sync.dma_start(bias_s[b][:], BIc.ap()[bass.ds(pid, 1), i])

            # ---- persistent SBUF ----
            vT512 = {b: persist.tile([128, N1], BF16, name=f"vT512_{b}",
                                     tag=f"vT512_{b}") for b in B}
            vTok = {b: persist.tile([128, KT, 128], BF16, name=f"vTok_{b}",
                                    tag=f"vTok_{b}") for b in B}
            kS = {b: persist.tile([128, KT, 128], BF16, name=f"kS_{b}",
                                  tag=f"kS_{b}") for b in B}
            vN = {b: persist.tile([128, KT, 128], BF16, name=f"vN_{b}",
                                  tag=f"vN_{b}") for b in B}
            qN = {b: persist.tile([128, N1], BF16, name=f"qN_{b}",
                                  tag=f"qN_{b}") for b in B}
            xs = {b: persist.tile([128, N1], BF16, name=f"xs_{b}",
                                  tag=f"xs_{b}") for b in B}
            P = {b: persist.tile([128, KT, N1], BF16, name=f"P_{b}",
                                 tag=f"P_{b}") for b in B}

            # ---------------- Phase A: head projections ----------------
            with tc.tile_pool(name="projw", bufs=1) as projw, \
                 tc.tile_pool(name="projx", bufs=2) as projx, \
                 tc.tile_pool(name="projtmp", bufs=2) as projtmp, \
                 tc.tile_pool(name="psA", bufs=3, space="PSUM") as psA, \
                 tc.tile_pool(name="psN", bufs=2, space="PSUM") as psN, \
                 tc.tile_pool(name="psT", bufs=2, space="PSUM") as psT:
                for ib, b in enumerate(B):
                    w_all = projw.tile([128, CC, 384], BF16, name="w_all",
                                       tag="w_all")
                    nc.sync.dma_start(w_all[:],
                                      WAc.ap()[bass.ds(pid, 1), ib].bitcast(BF16))

                    for tt in range(TT):
                        xt_t = projx.tile([128, CC, 512], BF16, name="xt", tag="xt")
                        nc.sync.dma_start(
                            xt_t[:], xa[ib][:, :, tt * 512:(tt + 1) * 512])

                        def proj(j, xt_t=xt_t, w_all=w_all):
                            ps = psA.tile([128, 512], F32, name="proj", tag="proj")
                            for c in range(CC):
                                nc.tensor.matmul(
                                    ps[:], w_all[:, c, j * 128:(j + 1) * 128],
                                    xt_t[:, c, :],
                                    start=(c == 0), stop=(c == CC - 1))
                            return ps

                        def inv_norm(ps):
                            sq = projtmp.tile([128, 512], BF16, name="sq", tag="sq")
                            nc.scalar.activation(sq[:], ps[:], AF.Square)
                            nsq = psN.tile([1, 512], F32, name="nsq", tag="nsq")
                            nc.tensor.matmul(nsq[:], ones[:], sq[:],
                                             start=True, stop=True)
                            st = projtmp.tile([1, 512], F32, name="st", tag="st")
                            nc.scalar.activation(st[:], nsq[:], AF.Sqrt)
                            rt = projtmp.tile([1, 512], F32, name="rt", tag="rt")
                            nc.vector.reciprocal(rt[:], st[:])
                            return rt

                        def bcast(row):
                            bt = projtmp.tile([128, 512], F32, name="bc", tag="bc")
                            nc.gpsimd.partition_broadcast(bt[:], row[:])
                            return bt

                        tsl = slice(tt * 4, (tt + 1) * 4)

                        # k: fold SCALE (and cls_score) and 1/|k| in
                        pk = proj(1)
                        rk = inv_norm(pk)
                        fk = projtmp.tile([1, 512], F32, name="fk", tag="fk")
                        nc.vector.tensor_scalar_mul(fk[:], rk[:], SCALE)
                        if b == "cls":
                            nc.vector.tensor_mul(
                                fk[:], fk[:], score_s[:, tt * 512:(tt + 1) * 512])
                        nc.vector.tensor_mul(kS[b][:, tsl, :], pk[:], bcast(fk)[:])

                        # v: normalized copy + raw copy + transposes
                        pv = proj(2)
                        rv = inv_norm(pv)
                        nc.vector.tensor_mul(vN[b][:, tsl, :], pv[:], bcast(rv)[:])
                        vraw = (vT512[b] if tt == 0 else
                                projtmp.tile([128, 512], BF16, name="vraw",
                                             tag="vraw"))
                        nc.scalar.activation(vraw[:], pv[:], AF.Copy)
                        for j in range(4):
                            tp = psT.tile([128, 128], BF16, name="tp", tag="tp")
                            nc.tensor.transpose(
                                tp[:], vraw[:, j * 128:(j + 1) * 128], ident[:])
                            nc.vector.tensor_copy(vTok[b][:, tt * 4 + j, :], tp[:])

                        # q (first token tile only)
                        if tt == 0:
                            pq = proj(0)
                            rq = inv_norm(pq)
                            nc.vector.tensor_mul(qN[b][:], pq[:], bcast(rq)[:])

            # ---------------- Phase B: attention + raw sims ----------------
            with tc.tile_pool(name="attnps", bufs=2, space="PSUM") as attnps, \
                 tc.tile_pool(name="rawps", bufs=2, space="PSUM") as rawps, \
                 tc.tile_pool(name="accps", bufs=1, space="PSUM") as accps, \
                 tc.tile_pool(name="attntmp", bufs=2) as attntmp, \
                 tc.tile_pool(name="rhpool", bufs=1) as rhpool, \
                 tc.tile_pool(name="avgpool", bufs=3) as avgpool:
                xacc = {b: accps.tile([128, N1], F32, name=f"x_{b}",
                                      tag=f"x_{b}") for b in B}
                dacc = {b: accps.tile([1, N1], F32, name=f"d_{b}",
                                      tag=f"d_{b}")[:] for b in B}
                for ib, b in enumerate(B):
                    for kt in range(KT):
                        s = attnps.tile([128, N1], F32, name="s", tag="s")
                        nc.tensor.matmul(s[:], kS[b][:, kt, :], qN[b][:],
                                         start=True, stop=True)
                        p_t = P[b][:, kt, :]
                        nc.scalar.activation(p_t, s[:], AF.Exp)
                        nc.tensor.matmul(dacc[b], ones[:], p_t,
                                         start=(kt == 0), stop=(kt == KT - 1))
                        # per-head raw v-v similarity for this key tile
                        rw = rawps.tile([128, N1], F32, name="rw", tag="rw")
                        nc.tensor.matmul(rw[:], vN[b][:, kt, :],
                                         vN[b][:, 0:4, :].rearrange(
                                             "p t n -> p (t n)"),
                                         start=True, stop=True)
                        rwb = avgpool.tile([128, N1], BF16, name="rwb", tag="rwb")
                        nc.scalar.activation(rwb[:], rw[:], AF.Copy)
                        r0 = (1 + ib) * N2 + kt * 128
                        nc.sync.dma_start(ar_in[r0:r0 + 128, :], rwb[:])

                Rhalf = {}
                for b in B:
                    d2 = attntmp.tile([1, N1], F32, name="d2", tag="d2")
                    nc.vector.tensor_scalar_mul(d2[:], dacc[b], 2.0)
                    rh = attntmp.tile([1, N1], F32, name="rh", tag="rh")
                    nc.vector.reciprocal(rh[:], d2[:])
                    Rhalf[b] = rhpool.tile([128, N1], F32, name=f"Rh_{b}",
                                           tag=f"Rh_{b}")
                    nc.gpsimd.partition_broadcast(Rhalf[b][:], rh[:])

                for kt in range(KT):
                    for b in B:
                        nc.vector.tensor_mul(P[b][:, kt, :], P[b][:, kt, :],
                                             Rhalf[b][:])
                    av = avgpool.tile([128, N1], BF16, name="avg", tag="avg")
                    nc.vector.tensor_add(av[:], P["cls"][:, kt, :],
                                         P["reg"][:, kt, :])
                    r0 = kt * 128
                    nc.sync.dma_start(ar_in[r0:r0 + 128, :], av[:])
                    for b in B:
                        for i2, b2 in enumerate(B):
                            nc.tensor.matmul(
                                xacc[b][:], vTok[b][:, kt, :], P[b2][:, kt, :],
                                start=(kt == 0 and i2 == 0),
                                stop=(kt == KT - 1 and i2 == 1))
                for b in B:
                    nc.scalar.activation(xs[b][:], xacc[b][:], AF.Copy)

            # ==== Phase C: output-linear partials -> tail of ar_in ====
            with tc.tile_pool(name="supw", bufs=1) as supw, \
                 tc.tile_pool(name="cps", bufs=3, space="PSUM") as cps, \
                 tc.tile_pool(name="ctmp", bufs=3) as ctmp:
                for ib, b in enumerate(B):
                    wl_s = supw.tile([128, 2, 16, 128], BF16, name=f"wl_{b}",
                                     tag=f"wl_{b}")
                    nc.sync.dma_start(wl_s[:],
                                      WLc.ap()[bass.ds(pid, 1), ib].bitcast(BF16))
                    for j in range(16):
                        op_ = cps.tile([128, N1], F32, name="op", tag="op")
                        nc.tensor.matmul(op_[:], wl_s[:, 0, j, :], xs[b][:],
                                         start=True, stop=False)
                        nc.tensor.matmul(op_[:], wl_s[:, 1, j, :], vT512[b][:],
                                         start=False, stop=True)
                        ob = ctmp.tile([128, N1], BF16, name="ob", tag="ob")
                        nc.scalar.activation(ob[:], op_[:], AF.Copy)
                        r0 = (3 + ib) * N2 + j * 128
                        nc.sync.dma_start(ar_in[r0:r0 + 128, :], ob[:])

            if not SHRINK_RS:
                nc.gpsimd.collective_compute(
                    "AllReduce", mybir.AluOpType.add, replica_groups=RG,
                    ins=[ar_in.opt()], outs=[ar_out.opt()])
            else:
                ar_s_in = dramp.tile([64, N1], BF16, name="ar_s_in")
                ar_s_out = dramp.tile([64, N1], BF16, name="ar_s_out")
                nc.sync.dma_start(ar_s_in[:], ar_in[0:64, :])
                nc.sync.dma_start(ar_out[:], ar_in[:])
                nc.gpsimd.collective_compute(
                    "AllReduce", mybir.AluOpType.add, replica_groups=RG,
                    ins=[ar_s_in.opt()], outs=[ar_s_out.opt()])
                nc.sync.dma_start(ar_out[0:64, :], ar_s_out[:])

            # ==== Phase D: masks + masked exp + this head's ave columns ====
            with tc.tile_pool(name="dpool", bufs=3) as dpool, \
                 tc.tile_pool(name="dsb", bufs=1) as dsb, \
                 tc.tile_pool(name="dps", bufs=1, space="PSUM") as dps, \
                 tc.tile_pool(name="ftmp", bufs=2) as ftmp:
                avacc = {b: dps.tile([128, N1], F32, name=f"av_{b}",
                                     tag=f"av_{b}") for b in B}
                dnum = {b: dps.tile([1, N1], F32, name=f"dn_{b}",
                                    tag=f"dn_{b}")[:] for b in B}
                for kt in range(KT):
                    asum = dpool.tile([128, N1], BF16, name="asum", tag="asum")
                    nc.sync.dma_start(asum[:], ar_out[kt * 128:(kt + 1) * 128, :])
                    rsc = dpool.tile([128, N1], BF16, name="rsc", tag="rsc")
                    r1 = N2 + kt * 128
                    nc.sync.dma_start(rsc[:], ar_out[r1:r1 + 128, :])
                    rsr = dpool.tile([128, N1], BF16, name="rsr", tag="rsr")
                    r2 = 2 * N2 + kt * 128
                    nc.sync.dma_start(rsr[:], ar_out[r2:r2 + 128, :])
                    e_t = dpool.tile([128, N1], BF16, name="e_t", tag="e_t")
                    nc.scalar.activation(e_t[:], asum[:], AF.Exp,
                                         scale=1.0 / N_CORES)
                    msk_c = dpool.tile([128, N1], BF16, name="mc", tag="mc")
                    nc.vector.tensor_scalar(
                        msk_c[:], rsc[:], 1.0 / N_CORES, 0.75,
                        mybir.AluOpType.mult, mybir.AluOpType.is_gt)
                    msk_o = dpool.tile([128, N1], BF16, name="mo", tag="mo")
                    nc.vector.tensor_scalar(
                        msk_o[:], rsr[:], 1.0 / N_CORES, 0.99,
                        mybir.AluOpType.mult, mybir.AluOpType.is_gt)
                    mes = dpool.tile([128, N1], BF16, name="mes", tag="mes")
                    nc.vector.tensor_mul(mes[:], e_t[:], msk_c[:])
                    meo = dpool.tile([128, N1], BF16, name="meo", tag="meo")
                    nc.vector.tensor_mul(meo[:], mes[:], msk_o[:])
                    mm = {"cls": mes, "reg": meo}
                    for b in B:
                        nc.tensor.matmul(avacc[b][:], vTok[b][:, kt, :],
                                         mm[b][:],
                                         start=(kt == 0), stop=(kt == KT - 1))
                        nc.tensor.matmul(dnum[b], ones[:], mm[b][:],
                                         start=(kt == 0), stop=(kt == KT - 1))

                # ave normalize + write; linear bias add + write
                for ib, b in enumerate(B):
                    rec = dsb.tile([1, N1], F32, name=f"rec_{b}", tag=f"rec_{b}")
                    nc.vector.reciprocal(rec[:], dnum[b])
                    Rd = ftmp.tile([128, N1], F32, name="Rd", tag="Rd")
                    nc.gpsimd.partition_broadcast(Rd[:], rec[:])
                    asb = ftmp.tile([128, N1], OUTDT, name="asb", tag="asb")
                    nc.vector.tensor_mul(asb[:], avacc[b][:], Rd[:])
                    nc.sync.dma_start(a_out[b], asb[:])

                    olt = dsb.tile([128, 2, N1], BF16, name=f"olt_{b}",
                                   tag=f"olt_{b}")
                    for m in range(2):
                        nc.sync.dma_start(
                            olt[:, m, :],
                            ar_out[bass.ds(pid * 256 + (3 + ib) * N2 + m * 128,
                                           128), :])
                    for m in range(2):
                        osb = ftmp.tile([128, N1], OUTDT, name="osb", tag="osb")
                        nc.vector.tensor_scalar_add(osb[:], olt[:, m, :],
                                                    bias_s[b][:, m:m + 1])
                        nc.sync.dma_start(o_out[b][m * 128:(m + 1) * 128, :],
                                          osb[:])

    nc.finalize()
    return nc


def make_in_maps(inputs: dict) -> list[dict]:
    x_cls = np.asarray(inputs["x_cls"], np.float32)[0]      # [N2, C]
    x_reg = np.asarray(inputs["x_reg"], np.float32)[0]
    score = np.asarray(inputs["cls_score"], np.float32).reshape(1, N2)
    xt_cls = _bf16_u16(x_cls.T)                             # [C, N2] u16
    xt_reg = _bf16_u16(x_reg.T)
    in_maps = []
    for h in range(N_CORES):
        hs = slice(h * HD, (h + 1) * HD)
        xin = np.concatenate([xt_cls[hs], xt_reg[hs]], 0)   # [256, N2]
        in_maps.append({"xin": xin, "score": score})
    return in_maps


def assemble(results: list[dict]) -> tuple[np.ndarray, np.ndarray]:
    feats = []
    for i, b in enumerate(B):
        ave = np.concatenate(
            [results[c]["out"][512 + i * 128:512 + (i + 1) * 128].T
             for c in range(N_CORES)], 1)
        out = np.concatenate(
            [results[c]["out"][i * 256:(i + 1) * 256].T
             for c in range(N_CORES)], 1)
        feats.append(np.concatenate([ave, out], 1).astype(np.float32))
    return feats[0], feats[1]


_CACHE = {}


def _const_key(inputs: dict) -> str:
    h = hashlib.sha256()
    for k in ("W_q_cls", "W_kv_cls", "W_q_reg", "W_kv_reg",
              "W_lin", "b_lin", "W_lin_reg", "b_lin_reg"):
        h.update(np.ascontiguousarray(np.asarray(inputs[k], np.float32)).tobytes())
    return h.hexdigest()


def get_nc(inputs: dict | None = None):
    if inputs is not None:
        key = _const_key(inputs)
        if _CACHE.get("key") != key:
            _CACHE.clear()
            _CACHE["key"] = key
            _CACHE["nc"] = build_nc(make_consts(inputs))
    return _CACHE["nc"]


class _Runner:
    """Cached jitted SPMD executor (mirrors bass2jax.run_bass_via_pjrt)."""

    def __init__(self, nc):
        import jax
        from jax.sharding import Mesh, PartitionSpec
        from jax.experimental.shard_map import shard_map
        from concourse.bass2jax import (_bass_exec_p, install_neuronx_cc_hook,
                                        partition_id_tensor)
        install_neuronx_cc_hook()
        self.jax = jax
        pname = nc.partition_id_tensor.name if nc.partition_id_tensor else None
        in_names, out_names, out_avals, zero_outs = [], [], [], []
        for alloc in nc.m.functions[0].allocations:
            if not isinstance(alloc, mybir.MemoryLocationSet):
                continue
            name = alloc.memorylocations[0].name
            if alloc.kind == "ExternalInput":
                if name != pname:
                    in_names.append(name)
            elif alloc.kind == "ExternalOutput":
                out_names.append(name)
                shape = tuple(alloc.tensor_shape)
                dtype = mybir.dt.np(alloc.dtype)
                out_avals.append(jax.core.ShapedArray(shape, dtype))
                zero_outs.append(np.zeros(shape, dtype))
        self.in_names, self.out_names = in_names, out_names
        self.out_avals, self.zero_outs = out_avals, zero_outs
        n_params, n_outs = len(in_names), len(out_names)
        all_in = in_names + out_names + ([pname] if pname else [])

        def _body(*args):
            operands = list(args)
            if pname is not None:
                operands.append(partition_id_tensor())
            return tuple(_bass_exec_p.bind(
                *operands, out_avals=tuple(out_avals), in_names=tuple(all_in),
                out_names=tuple(out_names), lowering_input_output_aliases=(),
                sim_require_finite=True, sim_require_nnan=True, nc=nc))

        devices = jax.devices()[:N_CORES]
        mesh = Mesh(np.asarray(devices), ("core",))
        self.fn = jax.jit(
            shard_map(_body, mesh=mesh,
                      in_specs=(PartitionSpec("core"),) * (n_params + n_outs),
                      out_specs=(PartitionSpec("core"),) * n_outs,
                      check_rep=False),
            keep_unused=True)

    def __call__(self, in_maps):
        n = N_CORES
        concat_in = [np.concatenate([np.asarray(in_maps[c][k]) for c in range(n)], 0)
                     for k in self.in_names]
        concat_zeros = [np.zeros((n * z.shape[0], *z.shape[1:]), z.dtype)
                        for z in self.zero_outs]
        outs = self.fn(*concat_in, *concat_zeros)
        self.jax.block_until_ready(outs)
        return [{name: np.asarray(outs[i]).reshape(n, *self.out_avals[i].shape)[c]
                 for i, name in enumerate(self.out_names)}
                for c in range(n)]


def get_runner(inputs: dict | None = None):
    nc = get_nc(inputs)
    if "runner" not in _CACHE:
        _CACHE["runner"] = _Runner(nc)
    return _CACHE["runner"]


def kernel(**inputs) -> tuple[np.ndarray, np.ndarray]:
    runner = get_runner(inputs)
    in_maps = make_in_maps(inputs)
    for _ in range(3):
        feats = assemble(runner(in_maps))
        if all(np.isfinite(f).all() for f in feats):
            return feats
    return feats
